# revision 30
# baseline (speedup 1.0000x reference)
"""Causal self-attention (B=2, T=2048, C=1024, H=16, D=64) on 8 TRN2 cores.

Sharding: core = 4*b + g  (b = batch, g = head-group of 4 heads).
Each core computes, for its batch b and heads 4g..4g+3:
  - qkv projection (column-parallel slice of w_qkv)
  - causal attention in transposed (S^T) space
  - row-parallel partial of the output projection (its 256 channels of C)
Host sums the 4 partial projections per batch and transposes back.

All matmuls run in float32r (TF32-class rounded fp32, 1 cyc/row on PE).
Attention is computed transposed: S_T[j,i] = k_j . q_i; the softmax
denominator (a partition-axis sum in this layout) is obtained by
augmenting the AV stationary operand with a column of ones, so it
accumulates for free in PSUM row 64. No max-subtraction is needed:
logits are O(sigma=1) by construction, exp cannot overflow.

Per 128-token j-block, both heads of a pair are packed: the two K=64
S matmuls run concurrently on distinct PE row-groups (partitions 0-63 /
64-127), write the two halves of one [128,1024] PSUM tile, and a single
ACTIVATE computes exp over both (the ~293ns fixed ACT cost per
instruction would otherwise dominate the softmax).
"""
import numpy as np

import concourse.bass as bass
import concourse.mybir as mybir
import concourse.tile as tile
from concourse import bacc
from concourse import bass_utils

B = 2
T = 2048
C = 1024
D = 64
HPG = 4          # heads per group (per core)
N_CORES = 8
TCH = 512        # token chunk (psum free width)
NT = T // TCH    # 4 chunks
NJ = T // 128    # 16 j-blocks
KT = C // 128    # 8 contraction tiles
F32 = mybir.dt.float32
F32R = mybir.dt.float32r
AF = mybir.ActivationFunctionType

_NC = None


def _make_upper_triangular(nc, out, val=1.0):
    """out[j, i] = val if j <= i else 0 (128x128)."""
    nc.gpsimd.memset(out, 0.0)
    nc.gpsimd.affine_select(
        out=out, in_=out,
        compare_op=mybir.AluOpType.is_gt,
        fill=val, base=0,
        pattern=[[-1, 128]],
        channel_multiplier=1,
    )


def _build():
    nc = bacc.Bacc(
        "TRN2", target_bir_lowering=False, debug=False, enable_asserts=False,
        num_devices=N_CORES,
    )
    # f32r-typed DRAM inputs: same bits as f32; lets DMA feed matmuls
    # directly (the PE's f32r path does its own mantissa handling).
    xT = nc.dram_tensor("xT", [C, T], F32R, kind="ExternalInput").ap()
    wqkT = nc.dram_tensor("wqkT", [C, 2 * HPG * D], F32R, kind="ExternalInput").ap()
    wvT = nc.dram_tensor("wvT", [C, HPG * D], F32R, kind="ExternalInput").ap()
    wpT = nc.dram_tensor("wpT", [HPG * D, C], F32R, kind="ExternalInput").ap()
    out = nc.dram_tensor("out", [C, T], F32, kind="ExternalOutput").ap()

    with tile.TileContext(nc) as tc:
        with tc.tile_pool(name="persist", bufs=1) as pers:
            # ---- persistent sbuf tensors
            wqkr = [pers.tile([128, 2 * HPG * D], F32R, name=f"wqkr{k}",
                              tag=f"wqkr{k}") for k in range(KT)]
            wvr = [pers.tile([128, HPG * D], F32R, name=f"wvr{k}", tag=f"wvr{k}")
                   for k in range(KT)]
            wpr = [pers.tile([128, C], F32R, name=f"wpr{k}", tag=f"wpr{k}")
                   for k in range(2)]
            qTr = [pers.tile([128, T], F32R, name=f"qTr{p}", tag=f"qTr{p}")
                   for p in range(2)]
            kTr = [pers.tile([128, T], F32R, name=f"kTr{p}", tag=f"kTr{p}")
                   for p in range(2)]
            yTr = qTr  # q chunk-c columns are dead once chunk-c S matmuls ran
            # v in natural layout, augmented with a ones column per head:
            # per j-block tt: [h0 v(64) | 1 | h1 v | 1 | h2 v | 1 | h3 v | 1]
            vaug = pers.tile([128, NJ * HPG * 65], F32R, name="vaug", tag="vaug")
            trimask = pers.tile([128, 128], F32, name="trimask", tag="trimask")
            # r=3 diagonal blocks run at width 256 (f32r matmuls are 4x slower
            # below N=256); mask3 zeroes the extra fully-invalid 128 columns.
            mask3 = pers.tile([128, 256], F32, name="mask3", tag="mask3")
            ones = pers.tile([128, NJ * HPG], F32, name="ones", tag="ones")

            # ---- long-lived working pools first so the phase-scoped pools
            # (xrpool/qkvps) sit on top of the pool stack and release LIFO.
            ptp = tc.alloc_tile_pool(name="ptp", bufs=8)
            nrm = tc.alloc_tile_pool(name="nrm", bufs=6)
            attps = tc.alloc_tile_pool(name="attps", bufs=2, space="PSUM")
            yps = tc.alloc_tile_pool(name="yps", bufs=1, space="PSUM")
            # ---- x tiles live only through the QKV phase
            xq = tc.alloc_tile_pool(name="xrpool", bufs=1)
            qkvps = tc.alloc_tile_pool(name="qkvps", bufs=1, space="PSUM")
            xr = [xq.tile([128, T], F32R, name=f"xr{k}", tag=f"xr{k}")
                  for k in range(KT)]

            # ---- loads, in consumption order (k-tiles of column 0 first).
            # DMA issue costs ~0.65us per descriptor on the issuing engine, so
            # spread issue across engines: x on gpsimd, weights on sync, the
            # later x columns on scalar (idle at start).
            for k in range(KT):
                nc.gpsimd.dma_start(
                    xr[k][:, 0:TCH], xT[128 * k:128 * (k + 1), 0:TCH])
                nc.scalar.dma_start(wvr[k][:], wvT[128 * k:128 * (k + 1), :])
            for k in range(KT):
                nc.sync.dma_start(wqkr[k][:], wqkT[128 * k:128 * (k + 1), :])
            for n in range(1, NT):
                for k in range(KT):
                    nc.gpsimd.dma_start(
                        xr[k][:, TCH * n:TCH * (n + 1)],
                        xT[128 * k:128 * (k + 1), TCH * n:TCH * (n + 1)])
            for k in range(2):
                nc.sync.dma_start(wpr[k][:], wpT[128 * k:128 * (k + 1), :])

            # ---- constants
            nc.gpsimd.memset(ones[:], 1.0)
            onescols = vaug.rearrange("p (t c) -> p t c", c=65)[:, :, 64]
            nc.vector.tensor_copy(onescols, ones[:])
            _make_upper_triangular(nc, trimask[:])
            nc.gpsimd.memset(mask3[:], 1.0)
            # mask3[j, i] = 1 iff i >= j + 128  (i in [0,256))
            nc.gpsimd.affine_select(
                out=mask3[:], in_=mask3[:],
                compare_op=mybir.AluOpType.is_ge,
                fill=0.0, base=-128,
                pattern=[[1, 256]],
                channel_multiplier=-1,
            )

            # ---- pipelined QKV columns + attention chunks.
            # Attention chunk c only reads q/k/v produced from x columns <= c,
            # so QKV(column n) and attention(chunk n-1) overlap: the PE-dense
            # projection matmuls fill the ACT-bound softmax stretches.
            # PSUM budget (8 banks): qk 2 + pv 2 + stg 2 + yacc 2.
            scale = float(D) ** -0.5
            if True:
                for c in range(NT):
                    # ---- QKV column c (v first: it only needs wv + x)
                    for t in range(4):
                        tt = 4 * c + t
                        pv = qkvps.tile([128, HPG * D], F32, name="pvp", tag="pv")
                        for k in range(KT):
                            nc.tensor.matmul(
                                pv[:],
                                xr[k][:, 128 * tt:128 * (tt + 1)],
                                wvr[k][:],
                                start=(k == 0), stop=(k == KT - 1))
                        dst = vaug[:, 260 * tt:260 * (tt + 1)].rearrange(
                            "p (h c) -> p h c", c=65)[:, :, 0:64]
                        nc.vector.tensor_copy(
                            dst, pv[:].rearrange("p (h c) -> p h c", c=64))
                    for m in range(4):
                        qk = qkvps.tile([128, TCH], F32, name="qkp", tag="qk")
                        for k in range(KT):
                            nc.tensor.matmul(
                                qk[:],
                                wqkr[k][:, 128 * m:128 * (m + 1)],
                                xr[k][:, TCH * c:TCH * (c + 1)],
                                start=(k == 0), stop=(k == KT - 1))
                        dest = (qTr if m < 2 else kTr)[m % 2]
                        nc.vector.tensor_copy(dest[:, TCH * c:TCH * (c + 1)], qk[:])

                    # ---- attention chunk c
                    for p in range(2):
                        yacc = [yps.tile([65, TCH], F32, name=f"yacc{h}",
                                         tag=f"y{h}") for h in range(2)]
                        njb = 4 * c + 4
                        for jb in range(njb):
                            diag_r = jb - 4 * c
                            i0 = 0 if diag_r < 0 else 128 * diag_r
                            if diag_r == 3:
                                i0 = 256  # keep matmul width >= 256 for f32r
                            # [j 128, h0-i 512 | h1-i 512]
                            stg = attps.tile([128, 2 * TCH], F32, name="stg",
                                             tag="stg")
                            ptg = ptp.tile([128, 2 * TCH], F32R, name="ptg",
                                           tag="ptg")
                            for h in range(2):
                                nc.tensor.matmul(
                                    stg[:, TCH * h + i0:TCH * (h + 1)],
                                    kTr[p][64 * h:64 * (h + 1),
                                           128 * jb:128 * (jb + 1)],
                                    qTr[p][64 * h:64 * (h + 1),
                                           TCH * c + i0:TCH * (c + 1)],
                                    start=True, stop=True)
                            if i0 == 0:
                                nc.scalar.activation(
                                    ptg[:], stg[:], AF.Exp, scale=scale)
                            else:
                                sv = stg.rearrange("p (h i) -> p h i",
                                                   i=TCH)[:, :, i0:TCH]
                                pv_ = ptg.rearrange("p (h i) -> p h i",
                                                    i=TCH)[:, :, i0:TCH]
                                nc.scalar.activation(
                                    pv_, sv, AF.Exp, scale=scale)
                            for h in range(2):
                                hg = 2 * p + h
                                if diag_r == 3:
                                    sl = slice(TCH * h + i0, TCH * (h + 1))
                                    nc.vector.tensor_mul(
                                        ptg[:, sl], ptg[:, sl].bitcast(F32),
                                        mask3[:])
                                elif diag_r >= 0:
                                    sl = slice(TCH * h + i0, TCH * h + i0 + 128)
                                    nc.vector.tensor_mul(
                                        ptg[:, sl], ptg[:, sl].bitcast(F32),
                                        trimask[:])
                                nc.tensor.matmul(
                                    yacc[h][:, i0:TCH],
                                    vaug[:, 260 * jb + 65 * hg:
                                         260 * jb + 65 * hg + 65],
                                    ptg[:, TCH * h + i0:TCH * (h + 1)],
                                    start=(jb == 0), stop=(jb == njb - 1))
                        for h in range(2):
                            rcp = nrm.tile([1, TCH], F32, name="rcp", tag="rcp")
                            nc.vector.reciprocal(rcp[:], yacc[h][64:65, :])
                            rb = nrm.tile([64, TCH], F32, name="rb", tag="rb")
                            nc.gpsimd.partition_broadcast(rb[:], rcp[:])
                            nc.vector.tensor_mul(
                                yTr[p][64 * h:64 * (h + 1), TCH * c:TCH * (c + 1)],
                                yacc[h][0:64, :], rb[:])

                    if c == NT - 1:
                        qkvps.release()
                        xq.release()

                # ---- output projection (fills PE gaps of the last, longest
                # attention chunk: yTr chunks 0..2 are ready before it runs)
                with (
                    tc.tile_pool(name="ostage", bufs=6) as ostage,
                    tc.tile_pool(name="pps", bufs=2, space="PSUM") as pps,
                ):
                    for n in range(NT):
                        for m in range(8):
                            pp = pps.tile([128, TCH], F32, name="pp", tag="pp")
                            for k in range(2):
                                nc.tensor.matmul(
                                    pp[:],
                                    wpr[k][:, 128 * m:128 * (m + 1)],
                                    yTr[k][:, TCH * n:TCH * (n + 1)],
                                    start=(k == 0), stop=(k == 1))
                            ob = ostage.tile([128, TCH], F32, name="ob", tag="ob")
                            nc.vector.tensor_copy(ob[:], pp[:])
                            nc.sync.dma_start(
                                out[128 * m:128 * (m + 1), TCH * n:TCH * (n + 1)],
                                ob[:])

            yps.release()
            attps.release()
            nrm.release()
            ptp.release()

    nc.compile()
    return nc


def _get_nc():
    global _NC
    if _NC is None:
        _NC = _build()
    return _NC


def _in_maps(x, w_qkv, w_proj):
    x = np.ascontiguousarray(np.asarray(x, dtype=np.float32))
    w_qkv = np.ascontiguousarray(np.asarray(w_qkv, dtype=np.float32))
    w_proj = np.ascontiguousarray(np.asarray(w_proj, dtype=np.float32))
    maps = []
    for core in range(N_CORES):
        b, g = divmod(core, 4)
        q_rows = w_qkv[256 * g:256 * (g + 1)]
        k_rows = w_qkv[C + 256 * g:C + 256 * (g + 1)]
        v_rows = w_qkv[2 * C + 256 * g:2 * C + 256 * (g + 1)]
        maps.append({
            "xT": np.ascontiguousarray(x[b].T),
            "wqkT": np.ascontiguousarray(np.concatenate([q_rows, k_rows], 0).T),
            "wvT": np.ascontiguousarray(v_rows.T),
            "wpT": np.ascontiguousarray(w_proj[:, 256 * g:256 * (g + 1)].T),
        })
    return maps


def _gather(results):
    full = np.empty((B, T, C), dtype=np.float32)
    for b in range(B):
        acc = results[4 * b]["out"].astype(np.float32).copy()
        for g in range(1, 4):
            acc += results[4 * b + g]["out"]
        full[b] = acc.T
    return full


def _run(x, w_qkv, w_proj, **kw):
    nc = _get_nc()
    res = bass_utils.run_bass_kernel_spmd(
        nc, _in_maps(x, w_qkv, w_proj), core_ids=list(range(N_CORES)), **kw)
    return _gather(res.results), res


def _run_in_subprocess(x, w_qkv, w_proj):
    """Clean-process fallback for transient device errors (a failed NRT exec
    poisons the in-process jax/axon state, so retry from scratch)."""
    import os
    import subprocess
    import sys
    import tempfile

    here = os.path.dirname(os.path.abspath(__file__))
    with tempfile.TemporaryDirectory() as td:
        np.savez(os.path.join(td, "in.npz"), x=x, w_qkv=w_qkv, w_proj=w_proj)
        prog = (
            "import os, numpy as np\n"
            f"os.chdir({td!r})\n"
            "import kernel\n"
            "d = np.load('in.npz')\n"
            "out = kernel.kernel(d['x'], d['w_qkv'], d['w_proj'])\n"
            "np.save('out.npy', out)\n"
        )
        env = dict(os.environ, KERNEL_NO_SUBPROCESS="1",
                   PYTHONPATH=here + os.pathsep + os.environ.get("PYTHONPATH", ""))
        last = None
        for attempt in range(2):
            try:
                subprocess.run([sys.executable, "-c", prog], env=env,
                               check=True, timeout=1200)
                return np.load(os.path.join(td, "out.npy"))
            except Exception as e:  # noqa: BLE001
                last = e
                import time
                time.sleep(20)
        raise last


def kernel(x, w_qkv, w_proj):
    import os
    try:
        out, _ = _run(x, w_qkv, w_proj)
        return out
    except Exception:
        if os.environ.get("KERNEL_NO_SUBPROCESS"):
            raise
        import time
        time.sleep(20)
        return _run_in_subprocess(np.asarray(x, np.float32),
                                  np.asarray(w_qkv, np.float32),
                                  np.asarray(w_proj, np.float32))
